# revision 27
# baseline (speedup 1.0000x reference)
"""GateRow kernel for Trainium2 (8 NeuronCores, SPMD, gate-sharded, bit-packed).

Problem: out[b, g] = gates[g, 2*x[b, c0[g]] + x[b, c1[g]]]
  x: [16384, 8192] bool, gates: [8192, 4] bool, choices: [8192, 2] int32.

Strategy:
  Every 2-input boolean gate is  rowA OP rowB  for OP in {AND, OR, XOR}
  once operand inversion and constants are absorbed into a doubled
  lookup table TAB = [x^T ; ~x^T ; ones ; zeros] (one row per wire).
  Bit-pack the batch dimension (8 rows/byte) so each TAB row is
  B/8 = 2048 bytes and the boolean op is a plain bitwise op (done on
  uint32 views: bitwise is byte-local, and 32-bit elements quarter the
  DVE element count).

  Shard by GATES: core k owns 1024 gates.  Host sorts gates into
  type-homogeneous blocks of 128 under a fixed per-core schedule
  (3 AND blocks, 3 OR blocks, 2 XOR blocks); "flexible" gates
  (constants / projections, expressible in any family) pad the
  buckets to exact capacity.  The host un-permutes output columns.

  Device (per core): dma_gathers (2048 rows, 2048 B/row, 4 MB total),
  8 stock tensor_tensor bitwise ops, 8 output DMAs (2 MB total).
  No PE, no PSUM, no custom DVE ops.
"""

import sys

for _p in ("/opt/trn_rl_repo", "/opt/pypackages"):
    if _p not in sys.path:
        sys.path.append(_p)

from contextlib import ExitStack

import numpy as np

import concourse.bass as bass
import concourse.bacc as bacc
import concourse.tile as tile
import concourse.mybir as mybir
from concourse import library_config
from concourse.bass_utils import run_bass_kernel_spmd

B, N, G, NCORES = 16384, 8192, 8192, 8
GPC = G // NCORES           # 1024 gates per core
NBLK = GPC // 128           # 8 gate blocks per core
PB = B // 8                 # 2048 packed bytes per table row
PW = PB // 4                # 512 packed uint32 words per table row
ROW_ONE = 2 * N             # all-ones table row
ROW_ZERO = 2 * N + 1        # all-zeros table row
NIND = 3                    # blocks gathered via indirect DMA (lib-load window)
G1, G2 = 3, 2               # blocks per dma_gather call (queues 1, 2)

# Per-core block op schedule: 3 AND, 3 OR, 2 XOR blocks of 128 gates.
SCHED = ("and",) * 3 + ("or",) * 3 + ("xor",) * 2
CAP = {"and": 3 * 128 * NCORES, "or": 3 * 128 * NCORES, "xor": 2 * 128 * NCORES}

# ---------------------------------------------------------------------------
# Gate classification.
#   tt bit (2a+b) = f(a, b).  Operand selectors:
#     0: x[c0]   1: ~x[c0]   2: x[c1]   3: ~x[c1]   4: ones   5: zeros
#   SEL[op][tt] = (selA, selB) with f == rowA op rowB; None if inexpressible.
# ---------------------------------------------------------------------------

_OPS = ("and", "or", "xor")
_NPOP = {"and": np.bitwise_and, "or": np.bitwise_or, "xor": np.bitwise_xor}


def _build_sel():
    sel = {op: [None] * 16 for op in _OPS}
    for tt in range(16):
        for op in _OPS:
            for sa in range(6):
                for sb in range(6):
                    ok = True
                    for a in (0, 1):
                        for b in (0, 1):
                            va = (a, 1 - a, b, 1 - b, 1, 0)[sa]
                            vb = (a, 1 - a, b, 1 - b, 1, 0)[sb]
                            r = int(_NPOP[op](va, vb))
                            if r != ((tt >> (2 * a + b)) & 1):
                                ok = False
                    if ok and sel[op][tt] is None:
                        sel[op][tt] = (sa, sb)
    return sel


_SEL = _build_sel()
_FAMS = [frozenset(op for op in _OPS if _SEL[op][tt] is not None) for tt in range(16)]


# ---------------------------------------------------------------------------
# Device program
# ---------------------------------------------------------------------------

_ALU = {
    "and": mybir.AluOpType.bitwise_and,
    "or": mybir.AluOpType.bitwise_or,
    "xor": mybir.AluOpType.bitwise_xor,
}


def build_nc(ncores=NCORES):
    """One SPMD program; all cores run it on their own gate shard."""
    nc = bacc.Bacc(
        "TRN2",
        target_bir_lowering=False,
        debug=False,
        num_devices=ncores,
        num_swdge_queues=4,
    )
    tab = nc.dram_tensor("tab", [2 * N + 2, PW], mybir.dt.uint32, kind="ExternalInput")
    # Offsets for the indirect-DMA blocks (0..NIND-1), int32 row per slot.
    idxs = nc.dram_tensor(
        "idxs", [128, NIND * 2], mybir.dt.int32, kind="ExternalInput"
    )
    # Wrapped int16 indices for the two dma_gather calls (blocks NIND..7).
    idxg = nc.dram_tensor(
        "idxg", [128, (G1 + G2) * 2 * 128 // 16], mybir.dt.int16,
        kind="ExternalInput",
    )
    outd = nc.dram_tensor("out", [GPC, PW], mybir.dt.uint32, kind="ExternalOutput")

    def emit_tt_out(bk, a_ap, b_ap, po, nc):
        o_t = po.tile([128, PW], mybir.dt.uint32, tag=f"o{bk}")
        nc.vector.tensor_tensor(o_t[:], a_ap, b_ap, op=_ALU[SCHED[bk]])
        # Alternate the two HWDGE rings (sync=qSPDynamicHW,
        # scalar=qActDynamicHW) so output writes run in parallel.
        eng = nc.sync if bk % 2 == 0 else nc.scalar
        eng.dma_start(outd[bk * 128 : (bk + 1) * 128, :], o_t[:])

    with tile.TileContext(nc) as tc, ExitStack() as ctx:
        pconst = ctx.enter_context(tc.tile_pool(name="const", bufs=1))
        pg = ctx.enter_context(tc.tile_pool(name="gather", bufs=1))
        po = ctx.enter_context(tc.tile_pool(name="osb", bufs=2))

        # Kick off the gpsimd mlp-library load (dma_gather ucode) in the
        # background; the indirect DMAs below don't need it.
        nc.gpsimd.load_library(library_config.mlp)

        idx_t = pconst.tile([128, NIND * 2], mybir.dt.int32)
        nc.sync.dma_start(idx_t[:], idxs[:])
        idxg_t = pconst.tile([128, idxg.shape[1]], mybir.dt.int16)
        nc.sync.dma_start(idxg_t[:], idxg[:])

        # Blocks 0..NIND-1 via native indirect DMAs (no ucode library):
        # one gathered row per partition per call.
        gts = []
        for s in range(2 * NIND):
            g_t = pg.tile([128, PW], mybir.dt.uint32, tag=f"g{s}")
            nc.gpsimd.indirect_dma_start(
                out=g_t[:],
                out_offset=None,
                in_=tab[:],
                in_offset=bass.IndirectOffsetOnAxis(
                    ap=idx_t[:, s : s + 1],
                    axis=0,
                ),
            )
            gts.append(g_t)
            if s % 2 == 1:
                bk = s // 2
                emit_tt_out(bk, gts[2 * bk][:], gts[2 * bk + 1][:], po, nc)

        # Blocks NIND..7 via two dma_gather calls on separate SWDGE queues
        # (desc-gen starts once the library load completes).
        pc1 = G1 * 2 * 128 // 16
        ga_t = pg.tile([128, 2 * G1, PW], mybir.dt.uint32, tag="ga")
        nc.gpsimd.dma_gather(
            ga_t[:], tab[:], idxg_t[:, :pc1],
            G1 * 2 * 128, G1 * 2 * 128, PW,
            single_packet=False, queue_num=1,
        )
        gb_t = pg.tile([128, 2 * G2, PW], mybir.dt.uint32, tag="gb")
        nc.gpsimd.dma_gather(
            gb_t[:], tab[:], idxg_t[:, pc1:],
            G2 * 2 * 128, G2 * 2 * 128, PW,
            single_packet=False, queue_num=2,
        )
        for j in range(G1):
            emit_tt_out(NIND + j, ga_t[:, 2 * j, :], ga_t[:, 2 * j + 1, :], po, nc)
        for j in range(G2):
            emit_tt_out(
                NIND + G1 + j, gb_t[:, 2 * j, :], gb_t[:, 2 * j + 1, :], po, nc
            )
    nc.compile()
    return nc


# ---------------------------------------------------------------------------
# Host-side input prep
# ---------------------------------------------------------------------------


def _prep(x, gates, choices):
    x8 = np.asarray(x, dtype=np.uint8)
    gates8 = np.asarray(gates, dtype=np.uint8)
    ch = np.asarray(choices, dtype=np.int64)

    # Packed doubled table (replicated on every core).
    xp = np.packbits(x8, axis=0)              # [B/8, N], bit MSB = lowest batch row
    tab = np.empty((2 * N + 2, PB), dtype=np.uint8)
    tab[:N] = xp.T
    tab[N : 2 * N] = ~tab[:N]
    tab[ROW_ONE] = 0xFF
    tab[ROW_ZERO] = 0x00
    tab32 = tab.view(np.uint32)

    # Bucket assignment: required-family gates first, flexible gates pad.
    tt = (gates8 << np.arange(4, dtype=np.uint8)).sum(axis=1).astype(np.int64)
    req = {op: [t for t in range(16) if _FAMS[t] == {op}] for op in _OPS}
    flex = [t for t in range(16) if len(_FAMS[t]) == 3]
    assert sum(len(v) for v in req.values()) + len(flex) == 16

    gid = np.arange(G)
    flex_pool = gid[np.isin(tt, flex)]
    fp = 0
    slots = {}
    for op in _OPS:
        need = gid[np.isin(tt, req[op])]
        pad = CAP[op] - len(need)
        assert pad >= 0, f"bucket {op} overflow: {len(need)} > {CAP[op]}"
        slots[op] = np.concatenate([need, flex_pool[fp : fp + pad]])
        fp += pad
    assert fp == len(flex_pool)

    # Device gate order (core-major, schedule-major) + operand row indices.
    npcg = {"and": 3 * 128, "or": 3 * 128, "xor": 2 * 128}
    perm = np.empty(G, dtype=np.int64)        # device row -> gate id
    ia = np.empty(G, dtype=np.int64)
    ib = np.empty(G, dtype=np.int64)
    r = 0
    for k in range(NCORES):
        for op in _OPS:
            g = slots[op][k * npcg[op] : (k + 1) * npcg[op]]
            lut = [_SEL[op][t] or (5, 5) for t in range(16)]  # (5,5) never used
            selA = np.array([s[0] for s in lut])[tt[g]]
            selB = np.array([s[1] for s in lut])[tt[g]]
            rows = np.stack(
                [ch[g, 0], ch[g, 0] + N, ch[g, 1], ch[g, 1] + N,
                 np.full(len(g), ROW_ONE), np.full(len(g), ROW_ZERO)]
            )
            n = len(g)
            perm[r : r + n] = g
            ia[r : r + n] = rows[selA, np.arange(n)]
            ib[r : r + n] = rows[selB, np.arange(n)]
            r += n
    assert r == G

    # Per-core index tensors.
    #  idxs (int32, [128, 2*NIND]): column 2*bk+w = tab row of gate
    #    (block bk, partition p), operand w -- indirect-DMA blocks.
    #  idxg (int16, wrapped): dma_gather rows for blocks NIND..NBLK-1,
    #    interleaved [a(blk), b(blk)] per call group, wrapped %16.
    def wrap(mat):                                # [rows, 128] -> [128, rows*8]
        flat = mat.reshape(-1).astype(np.int16)
        return np.tile(flat.reshape(-1, 16).T, (8, 1))

    in_maps = []
    for k in range(NCORES):
        s = slice(k * GPC, (k + 1) * GPC)
        iak = ia[s].reshape(NBLK, 128)
        ibk = ib[s].reshape(NBLK, 128)
        inter = np.empty((2 * NBLK, 128), dtype=np.int64)
        inter[0::2] = iak
        inter[1::2] = ibk
        idxs_np = np.ascontiguousarray(inter[: 2 * NIND].T.astype(np.int32))
        g1 = inter[2 * NIND : 2 * (NIND + G1)]
        g2 = inter[2 * (NIND + G1) :]
        idxg_np = np.ascontiguousarray(
            np.concatenate([wrap(g1), wrap(g2)], axis=1)
        )
        in_maps.append({"tab": tab32, "idxs": idxs_np, "idxg": idxg_np})
    return in_maps, perm


# ---------------------------------------------------------------------------
# Entry point
# ---------------------------------------------------------------------------

_NC_CACHE = {}


def _get_nc():
    if "nc" not in _NC_CACHE:
        _NC_CACHE["nc"] = build_nc()
    return _NC_CACHE["nc"]


def kernel(x, gates, choices):
    in_maps, perm = _prep(x, gates, choices)
    nc = _get_nc()
    res = run_bass_kernel_spmd(nc, in_maps, list(range(NCORES)))
    packed = np.concatenate(
        [res.results[k]["out"].view(np.uint8) for k in range(NCORES)], axis=0
    )
    ordered = np.empty_like(packed)
    ordered[perm] = packed                    # un-permute gate rows
    up = np.unpackbits(ordered, axis=1)       # [G, B] 0/1 uint8
    return up.view(np.bool_).T                # [B, G] bool view


# revision 29
# speedup vs baseline: 1.1840x; 1.1840x over previous
"""GateRow kernel for Trainium2 (8 NeuronCores, SPMD, gate-sharded, bit-packed).

Problem: out[b, g] = gates[g, 2*x[b, c0[g]] + x[b, c1[g]]]
  x: [16384, 8192] bool, gates: [8192, 4] bool, choices: [8192, 2] int32.

Strategy:
  Every 2-input boolean gate is  rowA OP rowB  for OP in {AND, OR, XOR}
  once operand inversion and constants are absorbed into a doubled
  lookup table TAB = [x^T ; ~x^T ; ones ; zeros] (one row per wire).
  Bit-pack the batch dimension (8 rows/byte) so each TAB row is
  B/8 = 2048 bytes and the boolean op is a plain bitwise op (done on
  uint32 views: bitwise is byte-local, and 32-bit elements quarter the
  DVE element count).

  Shard by GATES: core k owns 1024 gates.  Host sorts gates into
  type-homogeneous blocks of 128 under a fixed per-core schedule
  (3 AND blocks, 3 OR blocks, 2 XOR blocks); "flexible" gates
  (constants / projections, expressible in any family) pad the
  buckets to exact capacity.  The host un-permutes output columns.

  Device (per core): dma_gathers (2048 rows, 2048 B/row, 4 MB total),
  8 stock tensor_tensor bitwise ops, 8 output DMAs (2 MB total).
  No PE, no PSUM, no custom DVE ops.
"""

import sys

for _p in ("/opt/trn_rl_repo", "/opt/pypackages"):
    if _p not in sys.path:
        sys.path.append(_p)

from contextlib import ExitStack

import numpy as np

import concourse.bass as bass
import concourse.bacc as bacc
import concourse.tile as tile
import concourse.mybir as mybir
from concourse import library_config
from concourse.bass_utils import run_bass_kernel_spmd

B, N, G, NCORES = 16384, 8192, 8192, 8
GPC = G // NCORES           # 1024 gates per core
NBLK = GPC // 128           # 8 gate blocks per core
PB = B // 8                 # 2048 packed bytes per table row
PW = PB // 4                # 512 packed uint32 words per table row
ROW_ONE = 2 * N             # all-ones table row
ROW_ZERO = 2 * N + 1        # all-zeros table row
NCALLS = 4                  # dma_gather calls (a+b interleaved per call)

# Per-core block op schedule: 3 AND, 3 OR, 2 XOR blocks of 128 gates.
SCHED = ("and",) * 3 + ("or",) * 3 + ("xor",) * 2
CAP = {"and": 3 * 128 * NCORES, "or": 3 * 128 * NCORES, "xor": 2 * 128 * NCORES}

# ---------------------------------------------------------------------------
# Gate classification.
#   tt bit (2a+b) = f(a, b).  Operand selectors:
#     0: x[c0]   1: ~x[c0]   2: x[c1]   3: ~x[c1]   4: ones   5: zeros
#   SEL[op][tt] = (selA, selB) with f == rowA op rowB; None if inexpressible.
# ---------------------------------------------------------------------------

_OPS = ("and", "or", "xor")
_NPOP = {"and": np.bitwise_and, "or": np.bitwise_or, "xor": np.bitwise_xor}


def _build_sel():
    sel = {op: [None] * 16 for op in _OPS}
    for tt in range(16):
        for op in _OPS:
            for sa in range(6):
                for sb in range(6):
                    ok = True
                    for a in (0, 1):
                        for b in (0, 1):
                            va = (a, 1 - a, b, 1 - b, 1, 0)[sa]
                            vb = (a, 1 - a, b, 1 - b, 1, 0)[sb]
                            r = int(_NPOP[op](va, vb))
                            if r != ((tt >> (2 * a + b)) & 1):
                                ok = False
                    if ok and sel[op][tt] is None:
                        sel[op][tt] = (sa, sb)
    return sel


_SEL = _build_sel()
_FAMS = [frozenset(op for op in _OPS if _SEL[op][tt] is not None) for tt in range(16)]


# ---------------------------------------------------------------------------
# Device program
# ---------------------------------------------------------------------------

_ALU = {
    "and": mybir.AluOpType.bitwise_and,
    "or": mybir.AluOpType.bitwise_or,
    "xor": mybir.AluOpType.bitwise_xor,
}


def build_nc(ncalls=NCALLS, ncores=NCORES):
    """One SPMD program; all cores run it on their own gate shard.

    ncalls dma_gather calls; each gathers the A then B rows for
    NBLK/ncalls consecutive gate blocks (interleaved a,b per call group
    so compute on group i overlaps the gather of group i+1).
    """
    npc = NBLK // ncalls     # gate blocks per call group

    nc = bacc.Bacc(
        "TRN2",
        target_bir_lowering=False,
        debug=False,
        num_devices=ncores,
        num_swdge_queues=4,
    )
    tab = nc.dram_tensor("tab", [2 * N + 2, PW], mybir.dt.uint32, kind="ExternalInput")
    idxs = nc.dram_tensor(
        "idxs", [128, NBLK * 2], mybir.dt.int32, kind="ExternalInput"
    )
    outd = nc.dram_tensor("out", [GPC, PW], mybir.dt.uint32, kind="ExternalOutput")

    with tile.TileContext(nc) as tc, ExitStack() as ctx:
        pconst = ctx.enter_context(tc.tile_pool(name="const", bufs=1))
        pg = ctx.enter_context(tc.tile_pool(name="gather", bufs=1))
        po = ctx.enter_context(tc.tile_pool(name="osb", bufs=2))

        idx_t = pconst.tile([128, NBLK * 2], mybir.dt.int32)
        nc.sync.dma_start(idx_t[:], idxs[:])

        # Native indirect DMAs (no gpsimd ucode library): one row per
        # partition per call, offset idx_t[:, s] selects the tab row
        # landing in partition p of tile s.
        gts = []
        for s in range(2 * NBLK):
            g_t = pg.tile([128, PW], mybir.dt.uint32, tag=f"g{s}")
            bi = nc.gpsimd.indirect_dma_start(
                out=g_t[:],
                out_offset=None,
                in_=tab[:],
                in_offset=bass.IndirectOffsetOnAxis(
                    ap=idx_t[:, s : s + 1],
                    axis=0,
                ),
            )
            bi.ins.single_packet = True
            gts.append(g_t)
            if s % 2 == 1:
                bk = s // 2
                o_t = po.tile([128, PW], mybir.dt.uint32, tag=f"o{bk}")
                nc.vector.tensor_tensor(
                    o_t[:],
                    gts[2 * bk][:],
                    gts[2 * bk + 1][:],
                    op=_ALU[SCHED[bk]],
                )
                # Alternate the two HWDGE rings (sync=qSPDynamicHW,
                # scalar=qActDynamicHW) so output writes run in parallel.
                eng = nc.sync if bk % 2 == 0 else nc.scalar
                eng.dma_start(outd[bk * 128 : (bk + 1) * 128, :], o_t[:])
    nc.compile()
    return nc


# ---------------------------------------------------------------------------
# Host-side input prep
# ---------------------------------------------------------------------------


def _prep(x, gates, choices, ncalls=NCALLS):
    x8 = np.asarray(x, dtype=np.uint8)
    gates8 = np.asarray(gates, dtype=np.uint8)
    ch = np.asarray(choices, dtype=np.int64)

    # Packed doubled table (replicated on every core).
    xp = np.packbits(x8, axis=0)              # [B/8, N], bit MSB = lowest batch row
    tab = np.empty((2 * N + 2, PB), dtype=np.uint8)
    tab[:N] = xp.T
    tab[N : 2 * N] = ~tab[:N]
    tab[ROW_ONE] = 0xFF
    tab[ROW_ZERO] = 0x00
    tab32 = tab.view(np.uint32)

    # Bucket assignment: required-family gates first, flexible gates pad.
    tt = (gates8 << np.arange(4, dtype=np.uint8)).sum(axis=1).astype(np.int64)
    req = {op: [t for t in range(16) if _FAMS[t] == {op}] for op in _OPS}
    flex = [t for t in range(16) if len(_FAMS[t]) == 3]
    assert sum(len(v) for v in req.values()) + len(flex) == 16

    gid = np.arange(G)
    flex_pool = gid[np.isin(tt, flex)]
    fp = 0
    slots = {}
    for op in _OPS:
        need = gid[np.isin(tt, req[op])]
        pad = CAP[op] - len(need)
        assert pad >= 0, f"bucket {op} overflow: {len(need)} > {CAP[op]}"
        slots[op] = np.concatenate([need, flex_pool[fp : fp + pad]])
        fp += pad
    assert fp == len(flex_pool)

    # Device gate order (core-major, schedule-major) + operand row indices.
    npcg = {"and": 3 * 128, "or": 3 * 128, "xor": 2 * 128}
    perm = np.empty(G, dtype=np.int64)        # device row -> gate id
    ia = np.empty(G, dtype=np.int64)
    ib = np.empty(G, dtype=np.int64)
    r = 0
    for k in range(NCORES):
        for op in _OPS:
            g = slots[op][k * npcg[op] : (k + 1) * npcg[op]]
            lut = [_SEL[op][t] or (5, 5) for t in range(16)]  # (5,5) never used
            selA = np.array([s[0] for s in lut])[tt[g]]
            selB = np.array([s[1] for s in lut])[tt[g]]
            rows = np.stack(
                [ch[g, 0], ch[g, 0] + N, ch[g, 1], ch[g, 1] + N,
                 np.full(len(g), ROW_ONE), np.full(len(g), ROW_ZERO)]
            )
            n = len(g)
            perm[r : r + n] = g
            ia[r : r + n] = rows[selA, np.arange(n)]
            ib[r : r + n] = rows[selB, np.arange(n)]
            r += n
    assert r == G

    # Offset layout per core: offs[p, 2*bk + w] = tab row index of gate
    # (block bk, partition p), operand w -- matches tile slot (2j, 2j+1).
    in_maps = []
    for k in range(NCORES):
        s = slice(k * GPC, (k + 1) * GPC)
        iak = ia[s].reshape(NBLK, 128)
        ibk = ib[s].reshape(NBLK, 128)
        inter = np.empty((2 * NBLK, 128), dtype=np.int32)
        inter[0::2] = iak
        inter[1::2] = ibk
        idxs_np = np.ascontiguousarray(inter.T)   # [128, 2*NBLK]
        in_maps.append({"tab": tab32, "idxs": idxs_np})
    return in_maps, perm


# ---------------------------------------------------------------------------
# Entry point
# ---------------------------------------------------------------------------

_NC_CACHE = {}


def _get_nc():
    if "nc" not in _NC_CACHE:
        _NC_CACHE["nc"] = build_nc()
    return _NC_CACHE["nc"]


def kernel(x, gates, choices):
    in_maps, perm = _prep(x, gates, choices)
    nc = _get_nc()
    res = run_bass_kernel_spmd(nc, in_maps, list(range(NCORES)))
    packed = np.concatenate(
        [res.results[k]["out"].view(np.uint8) for k in range(NCORES)], axis=0
    )
    ordered = np.empty_like(packed)
    ordered[perm] = packed                    # un-permute gate rows
    up = np.unpackbits(ordered, axis=1)       # [G, B] 0/1 uint8
    return up.view(np.bool_).T                # [B, G] bool view


# revision 31
# speedup vs baseline: 1.3739x; 1.1603x over previous
"""GateRow kernel for Trainium2 (8 NeuronCores, SPMD, gate-sharded, bit-packed).

Problem: out[b, g] = gates[g, 2*x[b, c0[g]] + x[b, c1[g]]]
  x: [16384, 8192] bool, gates: [8192, 4] bool, choices: [8192, 2] int32.

Strategy:
  Every 2-input boolean gate is  rowA OP rowB  for OP in {AND, OR, XOR}
  once operand inversion and constants are absorbed into a doubled
  lookup table TAB = [x^T ; ~x^T ; ones ; zeros] (one row per wire).
  Bit-pack the batch dimension (8 rows/byte) so each TAB row is
  B/8 = 2048 bytes and the boolean op is a plain bitwise op (done on
  uint32 views: bitwise is byte-local, and 32-bit elements quarter the
  DVE element count).

  Shard by GATES: core k owns 1024 gates.  Host sorts gates into
  type-homogeneous blocks of 128.  "Flexible" gates (constants and
  projections, f == one table row) are concentrated into pure-COPY
  blocks that skip the second gather and the ALU entirely: the
  gathered tile is DMAed straight to the output.  The remaining
  blocks are one stock tensor_tensor bitwise op each.  The schedule
  (#and/#or/#xor/#copy blocks per core) is derived from the actual
  gate-type counts at kernel() time and compiled per schedule.
  The host un-permutes output columns.

  Gathers use native indirect DMAs (InstDMACopy with a row offset per
  partition) — no gpsimd ucode library load (~9 us saved), descriptor
  generation is the Q7 firmware at ~8.7 ns/row.
"""

import sys

for _p in ("/opt/trn_rl_repo", "/opt/pypackages"):
    if _p not in sys.path:
        sys.path.append(_p)

from contextlib import ExitStack

import numpy as np

import concourse.bass as bass
import concourse.bacc as bacc
import concourse.tile as tile
import concourse.mybir as mybir
from concourse.bass_utils import run_bass_kernel_spmd

B, N, G, NCORES = 16384, 8192, 8192, 8
GPC = G // NCORES           # 1024 gates per core
NBLK = GPC // 128           # 8 gate blocks per core
PB = B // 8                 # 2048 packed bytes per table row
PW = PB // 4                # 512 packed uint32 words per table row
ROW_ONE = 2 * N             # all-ones table row
ROW_ZERO = 2 * N + 1        # all-zeros table row

# ---------------------------------------------------------------------------
# Gate classification.
#   tt bit (2a+b) = f(a, b).  Operand selectors:
#     0: x[c0]   1: ~x[c0]   2: x[c1]   3: ~x[c1]   4: ones   5: zeros
#   _SEL[op][tt] = (selA, selB) with f == rowA op rowB; None if
#   inexpressible.  _SEL["copy"][tt] = (selA, selA) when f == rowA.
# ---------------------------------------------------------------------------

_OPS = ("and", "or", "xor")
_NPOP = {"and": np.bitwise_and, "or": np.bitwise_or, "xor": np.bitwise_xor}


def _val(sel, a, b):
    return (a, 1 - a, b, 1 - b, 1, 0)[sel]


def _build_sel():
    sel = {op: [None] * 16 for op in (*_OPS, "copy")}
    for tt in range(16):
        for sa in range(6):
            if all(
                _val(sa, a, b) == ((tt >> (2 * a + b)) & 1)
                for a in (0, 1) for b in (0, 1)
            ):
                sel["copy"][tt] = (sa, sa)
                break
        for op in _OPS:
            for sa in range(6):
                for sb in range(6):
                    ok = all(
                        int(_NPOP[op](_val(sa, a, b), _val(sb, a, b)))
                        == ((tt >> (2 * a + b)) & 1)
                        for a in (0, 1) for b in (0, 1)
                    )
                    if ok and sel[op][tt] is None:
                        sel[op][tt] = (sa, sb)
    return sel


_SEL = _build_sel()
# Required family per tt: the single op that expresses it, or "copy".
_REQ = [
    "copy" if _SEL["copy"][tt] is not None
    else next(op for op in _OPS if _SEL[op][tt] is not None)
    for tt in range(16)
]


# ---------------------------------------------------------------------------
# Device program (parameterized by the per-core block schedule)
# ---------------------------------------------------------------------------

_ALU = {
    "and": mybir.AluOpType.bitwise_and,
    "or": mybir.AluOpType.bitwise_or,
    "xor": mybir.AluOpType.bitwise_xor,
}


def build_nc(sched, ncores=NCORES):
    """One SPMD program; all cores run it on their own gate shard.

    sched: tuple of NBLK block kinds ("and"/"or"/"xor"/"copy").
    Copy blocks gather one row per gate and DMA it straight out; op
    blocks gather two rows and run one tensor_tensor bitwise op.
    """
    ncalls = sum(2 if k in _OPS else 1 for k in sched)

    nc = bacc.Bacc(
        "TRN2",
        target_bir_lowering=False,
        debug=False,
        num_devices=ncores,
        num_swdge_queues=4,
    )
    tab = nc.dram_tensor("tab", [2 * N + 2, PW], mybir.dt.uint32, kind="ExternalInput")
    idxs = nc.dram_tensor("idxs", [128, ncalls], mybir.dt.int32, kind="ExternalInput")
    outd = nc.dram_tensor("out", [GPC, PW], mybir.dt.uint32, kind="ExternalOutput")

    with tile.TileContext(nc) as tc, ExitStack() as ctx:
        pconst = ctx.enter_context(tc.tile_pool(name="const", bufs=1))
        pg = ctx.enter_context(tc.tile_pool(name="gather", bufs=1))
        po = ctx.enter_context(tc.tile_pool(name="osb", bufs=2))

        idx_t = pconst.tile([128, ncalls], mybir.dt.int32)
        nc.sync.dma_start(idx_t[:], idxs[:])

        def gather(s, tag):
            g_t = pg.tile([128, PW], mybir.dt.uint32, tag=tag)
            nc.gpsimd.indirect_dma_start(
                out=g_t[:],
                out_offset=None,
                in_=tab[:],
                in_offset=bass.IndirectOffsetOnAxis(ap=idx_t[:, s : s + 1], axis=0),
            )
            return g_t

        s = 0
        for bk, kind in enumerate(sched):
            # Alternate the two HWDGE rings (sync=qSPDynamicHW,
            # scalar=qActDynamicHW) so output writes run in parallel.
            eng = nc.sync if bk % 2 == 0 else nc.scalar
            osl = outd[bk * 128 : (bk + 1) * 128, :]
            if kind == "copy":
                a_t = gather(s, f"g{s}")
                s += 1
                eng.dma_start(osl, a_t[:])
            else:
                a_t = gather(s, f"g{s}")
                b_t = gather(s + 1, f"g{s + 1}")
                s += 2
                o_t = po.tile([128, PW], mybir.dt.uint32, tag=f"o{bk}")
                nc.vector.tensor_tensor(o_t[:], a_t[:], b_t[:], op=_ALU[kind])
                eng.dma_start(osl, o_t[:])
        assert s == ncalls
    nc.compile()
    return nc


# ---------------------------------------------------------------------------
# Host-side input prep
# ---------------------------------------------------------------------------


def _prep(x, gates, choices):
    x8 = np.asarray(x, dtype=np.uint8)
    gates8 = np.asarray(gates, dtype=np.uint8)
    ch = np.asarray(choices, dtype=np.int64)

    # Packed doubled table (replicated on every core).
    xp = np.packbits(x8, axis=0)              # [B/8, N], bit MSB = lowest batch row
    tab = np.empty((2 * N + 2, PB), dtype=np.uint8)
    tab[:N] = xp.T
    tab[N : 2 * N] = ~tab[:N]
    tab[ROW_ONE] = 0xFF
    tab[ROW_ZERO] = 0x00
    tab32 = tab.view(np.uint32)

    # Data-driven schedule: block counts from the actual type census.
    tt = (gates8 << np.arange(4, dtype=np.uint8)).sum(axis=1).astype(np.int64)
    req = np.array([_REQ[t] for t in range(16)])[tt]    # per-gate family
    gid = np.arange(G)
    nblk = {op: -(-int((req == op).sum()) // (128 * NCORES)) for op in _OPS}
    bcopy = NBLK - sum(nblk.values())
    assert bcopy >= 0, f"schedule overflow: {nblk}"
    sched = sum(((op,) * nblk[op] for op in _OPS), ()) + ("copy",) * bcopy
    cap = {op: nblk[op] * 128 * NCORES for op in _OPS}
    cap["copy"] = bcopy * 128 * NCORES

    # Fill op buckets with their required gates, pad with copy-capable
    # gates; remaining copy gates fill the copy blocks exactly.
    flex_pool = gid[req == "copy"]
    fp = 0
    slots = {}
    for op in _OPS:
        need = gid[req == op]
        pad = cap[op] - len(need)
        assert pad >= 0
        slots[op] = np.concatenate([need, flex_pool[fp : fp + pad]])
        fp += pad
    slots["copy"] = flex_pool[fp:]
    assert len(slots["copy"]) == cap["copy"]

    # Device gate order (core-major, schedule-major) + operand rows.
    npc = {k: nblk.get(k, bcopy) * 128 for k in (*_OPS, "copy")}
    ncalls = sum(2 if k in _OPS else 1 for k in sched)
    perm = np.empty(G, dtype=np.int64)        # device row -> gate id
    offs = np.empty((NCORES, 128, ncalls), dtype=np.int32)
    r = 0
    for k in range(NCORES):
        s = 0
        for op in (*_OPS, "copy"):
            g = slots[op][k * npc[op] : (k + 1) * npc[op]]
            lut = [_SEL[op][t] or (5, 5) for t in range(16)]
            selA = np.array([q[0] for q in lut])[tt[g]]
            selB = np.array([q[1] for q in lut])[tt[g]]
            rows = np.stack(
                [ch[g, 0], ch[g, 0] + N, ch[g, 1], ch[g, 1] + N,
                 np.full(len(g), ROW_ONE), np.full(len(g), ROW_ZERO)]
            )
            n = len(g)
            perm[r : r + n] = g
            ra = rows[selA, np.arange(n)].reshape(-1, 128)
            rb = rows[selB, np.arange(n)].reshape(-1, 128)
            for j in range(n // 128):
                offs[k, :, s] = ra[j]
                s += 1
                if op != "copy":
                    offs[k, :, s] = rb[j]
                    s += 1
            r += n
        assert s == ncalls
    assert r == G

    in_maps = [
        {"tab": tab32, "idxs": np.ascontiguousarray(offs[k])} for k in range(NCORES)
    ]
    return in_maps, perm, sched


# ---------------------------------------------------------------------------
# Entry point
# ---------------------------------------------------------------------------

_NC_CACHE = {}


def _get_nc(sched):
    if sched not in _NC_CACHE:
        _NC_CACHE[sched] = build_nc(sched)
    return _NC_CACHE[sched]


def kernel(x, gates, choices):
    in_maps, perm, sched = _prep(x, gates, choices)
    nc = _get_nc(sched)
    res = run_bass_kernel_spmd(nc, in_maps, list(range(NCORES)))
    packed = np.concatenate(
        [res.results[k]["out"].view(np.uint8) for k in range(NCORES)], axis=0
    )
    ordered = np.empty_like(packed)
    ordered[perm] = packed                    # un-permute gate rows
    up = np.unpackbits(ordered, axis=1)       # [G, B] 0/1 uint8
    return up.view(np.bool_).T                # [B, G] bool view
